# revision 13
# baseline (speedup 1.0000x reference)
"""PE-one-hot variant: relation planes expanded by TensorE matmuls from an
SBUF-resident table instead of 1024 per-element HBM row gathers.

Same math as kernel.py; the [500 x (phi|m)] table stays in SBUF as a
4-chunk rhs (relation 125c+p at partition p, chunk c).  Per tile the
one-hot matrix oh[p,b] = (relidx[b] == 125c+p) is built with 4 Pool
tensor_scalar compares, and two 4-matmul accumulation groups produce
phi / m planes for the tile's 128 elements in PSUM (exact: one-hot).
"""

import sys

for _p in ("/opt/trn_rl_repo",):
    if _p not in sys.path:
        sys.path.insert(0, _p)

import numpy as np

import concourse.bass as bass
import concourse.bacc as bacc
import concourse.tile as tile
from concourse import mybir
from concourse.bass_utils import run_bass_kernel_spmd

NENTITY, NRELATION, EMB_DIM, K = 200000, 500, 512, 2
BATCH = 8192
NCORES = 8
B_LOC = BATCH // NCORES
NT = B_LOC // 128
CDT = mybir.dt.float16
NP_CDT = np.float16

F32 = mybir.dt.float32
F8 = mybir.dt.float8e4
I16 = mybir.dt.int16
I32 = mybir.dt.int32
AF = mybir.ActivationFunctionType
ALU = mybir.AluOpType

REL_P = 125
NCHUNK = 4
RX_FREE = NCHUNK * EMB_DIM  # 2048, chunk-major relation layout


def build_program():
    nc = bacc.Bacc("TRN2", target_bir_lowering=False, debug=False,
                   num_swdge_queues=4)

    ea = nc.declare_dram_parameter("ea", [NENTITY, 2 * EMB_DIM], CDT, isOutput=False)
    relx = nc.declare_dram_parameter("relx", [REL_P, RX_FREE], F8, isOutput=False)
    rely = nc.declare_dram_parameter("rely", [REL_P, RX_FREE], F8, isOutput=False)
    alphaf = nc.declare_dram_parameter("alphaf", [REL_P, RX_FREE], F8, isOutput=False)
    htidx = nc.declare_dram_parameter("htidx", [128, 2 * NT], I32, isOutput=False)
    relb = nc.declare_dram_parameter("relb", [128, B_LOC], CDT, isOutput=False)
    iotat = nc.declare_dram_parameter("iotat", [128, NCHUNK], F32, isOutput=False)
    out = nc.declare_dram_parameter("out", [128, NT], F32, isOutput=True)

    with tile.TileContext(nc) as tc:
        with (
            tc.tile_pool(name="idx", bufs=1) as idxp,
            tc.tile_pool(name="prep", bufs=1) as prep,
            tc.tile_pool(name="gat", bufs=8) as gat,
            tc.tile_pool(name="oh", bufs=2) as ohp,
            tc.tile_pool(name="psum", bufs=4, space="PSUM") as psp,
            tc.tile_pool(name="wrk", bufs=2) as wrk,
            tc.tile_pool(name="outp", bufs=1) as outp,
        ):
            # ---- uploads (relation tables first; ht index last on the
            # sync ring so gather payload can't starve them)
            alsb = prep.tile([REL_P, RX_FREE], F8)
            nc.scalar.dma_start(out=alsb[:], in_=alphaf[:])
            relxb = prep.tile([REL_P, RX_FREE], F8)
            nc.sync.dma_start(out=relxb[:], in_=relx[:])
            relyb = prep.tile([REL_P, RX_FREE], F8)
            nc.scalar.dma_start(out=relyb[:], in_=rely[:])
            relbt = idxp.tile([128, B_LOC], CDT)
            nc.scalar.dma_start(out=relbt[:], in_=relb[:])
            iott = idxp.tile([128, NCHUNK], F32)
            nc.scalar.dma_start(out=iott[:], in_=iotat[:])
            ht_t = idxp.tile([128, 2 * NT], I32)
            nc.sync.dma_start(out=ht_t[:], in_=htidx[:])

            # ---- entity gathers ---------------------------------------
            def igather(out_ap, in_ap, off_ap):
                return nc.gpsimd.indirect_dma_start(
                    out=out_ap, out_offset=None, in_=in_ap,
                    in_offset=bass.IndirectOffsetOnAxis(ap=off_ap, axis=0),
                )

            ghts = []
            for t in range(NT):
                ght = gat.tile([128, 2048], CDT, tag="ght")
                igather(ght[:, 0:1024], ea[:], ht_t[:, 2 * t : 2 * t + 1])
                igather(ght[:, 1024:2048], ea[:], ht_t[:, 2 * t + 1 : 2 * t + 2])
                ghts.append(ght)

            # ---- relation-plane table in SBUF -------------------------
            # rhs[p, c, 0:512] = phi for relation 125c+p
            # rhs[p, c, 512:1024] = m   for relation 125c+p
            rhs = prep.tile([128, NCHUNK, 2 * EMB_DIM], CDT)
            nc.vector.memset(rhs[:], 0.0)
            asg = prep.tile([REL_P, RX_FREE], CDT)
            nc.scalar.activation(asg[:], alsb[:], AF.Sign)
            xy = prep.tile([REL_P, RX_FREE], CDT)
            nc.vector.tensor_tensor(out=xy[:], in0=relxb[:], in1=relyb[:],
                                    op=ALU.mult)
            sxy = prep.tile([REL_P, RX_FREE], CDT)
            nc.scalar.activation(sxy[:], xy[:], AF.Sign)
            a3 = asg[:].rearrange("p (c d) -> p c d", d=EMB_DIM)
            s3 = sxy[:].rearrange("p (c d) -> p c d", d=EMB_DIM)
            xy3 = xy[:].rearrange("p (c d) -> p c d", d=EMB_DIM)
            nc.vector.tensor_tensor(
                out=rhs[0:REL_P, :, 0:EMB_DIM], in0=a3, in1=s3, op=ALU.mult
            )
            nc.vector.tensor_scalar(
                out=rhs[0:REL_P, :, EMB_DIM : 2 * EMB_DIM],
                in0=xy3, scalar1=0.0, scalar2=None, op0=ALU.is_gt,
            )

            # ---- main loop --------------------------------------------
            scores = outp.tile([128, NT], F32)
            junk = outp.tile([128, EMB_DIM], CDT)

            for t in range(NT):
                ght = ghts[t]
                h1 = ght[:, 0:512]
                h0 = ght[:, 512:1024]
                t1 = ght[:, 1024:1536]
                t0 = ght[:, 1536:2048]

                oh = ohp.tile([128, NCHUNK, 128], CDT, tag="oh")
                for c in range(NCHUNK):
                    nc.gpsimd.tensor_scalar(
                        out=oh[:, c, :],
                        in0=relbt[:, 128 * t : 128 * (t + 1)],
                        scalar1=iott[:, c : c + 1], scalar2=None,
                        op0=ALU.is_equal,
                    )
                php = psp.tile([128, EMB_DIM], F32, tag="php")
                mmp = psp.tile([128, EMB_DIM], F32, tag="mmp")
                for c in range(NCHUNK):
                    nc.tensor.matmul(
                        php[:], oh[:, c, :], rhs[:, c, 0:EMB_DIM],
                        start=(c == 0), stop=(c == NCHUNK - 1),
                    )
                for c in range(NCHUNK):
                    nc.tensor.matmul(
                        mmp[:], oh[:, c, :], rhs[:, c, EMB_DIM : 2 * EMB_DIM],
                        start=(c == 0), stop=(c == NCHUNK - 1),
                    )

                mskf = wrk.tile([128, EMB_DIM], CDT, tag="mskf")
                nc.scalar.activation(mskf[:], mmp[:], AF.Copy)
                uv1 = wrk.tile([128, 2, EMB_DIM], CDT, tag="uv1")
                nc.vector.tensor_tensor(out=uv1[:, 0, :], in0=h0, in1=t0,
                                        op=ALU.mult)
                nc.vector.tensor_tensor(out=uv1[:, 1, :], in0=h1, in1=t0,
                                        op=ALU.mult)
                pt1 = wrk.tile([128, EMB_DIM], CDT, tag="pt1")
                nc.vector.tensor_tensor(out=pt1[:], in0=php[:], in1=t1,
                                        op=ALU.mult)
                w2 = wrk.tile([128, 2, EMB_DIM], CDT, tag="w2")
                pt1b = pt1[:].rearrange("p (o d) -> p o d", o=1).broadcast_to(
                    [128, 2, EMB_DIM]
                )
                nc.vector.tensor_tensor(
                    out=w2[:],
                    in0=ght[:, 0:1024].rearrange("p (o d) -> p o d", d=EMB_DIM),
                    in1=pt1b, op=ALU.mult,
                )
                uv = wrk.tile([128, 2, EMB_DIM], CDT, tag="uv")
                nc.vector.tensor_tensor(out=uv[:], in0=uv1[:], in1=w2[:],
                                        op=ALU.add)
                nc.vector.copy_predicated(uv[:, 1, :], mskf[:].bitcast(I16),
                                          uv[:, 0, :])
                nc.scalar.activation(
                    junk[:], uv[:, 1, :], AF.Square,
                    accum_out=scores[:, t : t + 1],
                )

            res = outp.tile([128, NT], F32)
            nc.scalar.activation(res[:], scores[:], AF.Sqrt)
            nc.sync.dma_start(out=out[:], in_=res[:])

    nc.compile()
    return nc


_NC_CACHE = None


def _get_program():
    global _NC_CACHE
    if _NC_CACHE is None:
        _NC_CACHE = build_program()
    return _NC_CACHE


def make_in_maps(head_idx, relation_idx, tail_idx, entity_embedding,
                 relation_embedding, alpha_embedding):
    import ml_dtypes
    NP_F8 = ml_dtypes.float8_e4m3

    head_idx = np.asarray(head_idx).astype(np.int32)
    relation_idx = np.asarray(relation_idx).astype(np.int32)
    tail_idx = np.asarray(tail_idx).astype(np.int32)
    ent = np.asarray(entity_embedding)
    rel = np.asarray(relation_embedding)
    alp = np.asarray(alpha_embedding)

    ea = np.ascontiguousarray(
        ent[:, :, 0, ::-1].transpose(0, 2, 1).reshape(NENTITY, 2 * EMB_DIM)
    ).astype(NP_CDT)

    def chunkmajor(a):  # [500, 512] -> [125, 4*512], relation 125c+p at (p, c)
        return np.ascontiguousarray(
            a.reshape(NCHUNK, REL_P, EMB_DIM).transpose(1, 0, 2)
        ).reshape(REL_P, RX_FREE)

    relx = chunkmajor(rel[:, :, 0] * 128.0).astype(NP_F8)
    rely = chunkmajor(rel[:, :, 1] * 128.0).astype(NP_F8)
    alphaf = chunkmajor(alp * 128.0).astype(NP_F8)

    iota = np.full((128, NCHUNK), -1, np.float32)
    for c in range(NCHUNK):
        iota[0:REL_P, c] = 125 * c + np.arange(REL_P)

    in_maps = []
    for c in range(NCORES):
        lo = c * B_LOC
        h = head_idx[lo : lo + B_LOC]
        tl = tail_idx[lo : lo + B_LOC]
        r = relation_idx[lo : lo + B_LOC]
        htp = np.empty((128, 2 * NT), np.int32)
        for t in range(NT):
            htp[:, 2 * t] = h[128 * t : 128 * (t + 1)]
            htp[:, 2 * t + 1] = tl[128 * t : 128 * (t + 1)]
        relbv = np.tile(r.astype(np.float16)[None, :], (128, 1))
        in_maps.append(
            {
                "ea": ea,
                "relx": relx,
                "rely": rely,
                "alphaf": alphaf,
                "htidx": htp,
                "relb": relbv,
                "iotat": iota,
            }
        )
    return in_maps


def unshard_out(results):
    full = np.empty(BATCH, np.float32)
    for c in range(NCORES):
        o = np.asarray(results[c]["out"])
        full[c * B_LOC : (c + 1) * B_LOC] = o.T.ravel()
    return full


def kernel(head_idx, relation_idx, tail_idx, entity_embedding,
           relation_embedding, alpha_embedding):
    nc = _get_program()
    in_maps = make_in_maps(head_idx, relation_idx, tail_idx, entity_embedding,
                           relation_embedding, alpha_embedding)
    res = run_bass_kernel_spmd(nc, in_maps, list(range(NCORES)))
    return unshard_out(res.results)


# revision 14
# speedup vs baseline: 2.3637x; 2.3637x over previous
"""PE-one-hot variant: relation planes expanded by TensorE matmuls from an
SBUF-resident table instead of 1024 per-element HBM row gathers.

Same math as kernel.py; the [500 x (phi|m)] table stays in SBUF as a
4-chunk rhs (relation 125c+p at partition p, chunk c).  Per tile the
one-hot matrix oh[p,b] = (relidx[b] == 125c+p) is built with 4 Pool
tensor_scalar compares, and two 4-matmul accumulation groups produce
phi / m planes for the tile's 128 elements in PSUM (exact: one-hot).
"""

import sys

for _p in ("/opt/trn_rl_repo",):
    if _p not in sys.path:
        sys.path.insert(0, _p)

import numpy as np

import concourse.bass as bass
import concourse.bacc as bacc
import concourse.tile as tile
from concourse import mybir
from concourse.bass_utils import run_bass_kernel_spmd

NENTITY, NRELATION, EMB_DIM, K = 200000, 500, 512, 2
BATCH = 8192
NCORES = 8
B_LOC = BATCH // NCORES
NT = B_LOC // 128
CDT = mybir.dt.float16
NP_CDT = np.float16

F32 = mybir.dt.float32
F8 = mybir.dt.float8e4
I16 = mybir.dt.int16
I32 = mybir.dt.int32
AF = mybir.ActivationFunctionType
ALU = mybir.AluOpType

REL_P = 125
NCHUNK = 4
RX_FREE = NCHUNK * EMB_DIM  # 2048, chunk-major relation layout


def build_program():
    nc = bacc.Bacc("TRN2", target_bir_lowering=False, debug=False,
                   num_swdge_queues=4)

    ea = nc.declare_dram_parameter("ea", [NENTITY, 2 * EMB_DIM], CDT, isOutput=False)
    relx = nc.declare_dram_parameter("relx", [REL_P, RX_FREE], F8, isOutput=False)
    rely = nc.declare_dram_parameter("rely", [REL_P, RX_FREE], F8, isOutput=False)
    alphaf = nc.declare_dram_parameter("alphaf", [REL_P, RX_FREE], F8, isOutput=False)
    htidx = nc.declare_dram_parameter("htidx", [128, 2 * NT], I32, isOutput=False)
    ohtab = nc.declare_dram_parameter("ohtab", [128, NT * NCHUNK * 128], F8, isOutput=False)
    out = nc.declare_dram_parameter("out", [128, NT], F32, isOutput=True)

    with tile.TileContext(nc) as tc:
        with (
            tc.tile_pool(name="idx", bufs=1) as idxp,
            tc.tile_pool(name="prep", bufs=1) as prep,
            tc.tile_pool(name="gat", bufs=8) as gat,
            tc.tile_pool(name="oh", bufs=2) as ohp,
            tc.tile_pool(name="psum", bufs=4, space="PSUM") as psp,
            tc.tile_pool(name="wrk", bufs=2) as wrk,
            tc.tile_pool(name="outp", bufs=1) as outp,
        ):
            # ---- uploads (relation tables first; ht index last on the
            # sync ring so gather payload can't starve them)
            alsb = prep.tile([REL_P, RX_FREE], F8)
            nc.scalar.dma_start(out=alsb[:], in_=alphaf[:])
            relxb = prep.tile([REL_P, RX_FREE], F8)
            nc.sync.dma_start(out=relxb[:], in_=relx[:])
            relyb = prep.tile([REL_P, RX_FREE], F8)
            nc.scalar.dma_start(out=relyb[:], in_=rely[:])
            oht = idxp.tile([128, NT, NCHUNK, 128], F8)
            nc.scalar.dma_start(
                out=oht[:].rearrange("p t c b -> p (t c b)"), in_=ohtab[:]
            )
            ht_t = idxp.tile([128, 2 * NT], I32)
            nc.sync.dma_start(out=ht_t[:], in_=htidx[:])

            # ---- entity gathers ---------------------------------------
            def igather(out_ap, in_ap, off_ap):
                return nc.gpsimd.indirect_dma_start(
                    out=out_ap, out_offset=None, in_=in_ap,
                    in_offset=bass.IndirectOffsetOnAxis(ap=off_ap, axis=0),
                )

            ghts = []
            for t in range(NT):
                ght = gat.tile([128, 2048], CDT, tag="ght")
                igather(ght[:, 0:1024], ea[:], ht_t[:, 2 * t : 2 * t + 1])
                igather(ght[:, 1024:2048], ea[:], ht_t[:, 2 * t + 1 : 2 * t + 2])
                ghts.append(ght)

            # ---- relation-plane table in SBUF -------------------------
            # rhs[p, c, 0:512] = phi for relation 125c+p
            # rhs[p, c, 512:1024] = m   for relation 125c+p
            rhs = prep.tile([128, NCHUNK, 2 * EMB_DIM], F8)
            nc.vector.memset(rhs[:], 0.0)
            asg = prep.tile([REL_P, RX_FREE], CDT)
            nc.scalar.activation(asg[:], alsb[:], AF.Sign)
            xy = prep.tile([REL_P, RX_FREE], CDT)
            nc.vector.tensor_tensor(out=xy[:], in0=relxb[:], in1=relyb[:],
                                    op=ALU.mult)
            sxy = prep.tile([REL_P, RX_FREE], CDT)
            nc.scalar.activation(sxy[:], xy[:], AF.Sign)
            a3 = asg[:].rearrange("p (c d) -> p c d", d=EMB_DIM)
            s3 = sxy[:].rearrange("p (c d) -> p c d", d=EMB_DIM)
            xy3 = xy[:].rearrange("p (c d) -> p c d", d=EMB_DIM)
            nc.vector.tensor_tensor(
                out=rhs[0:REL_P, :, 0:EMB_DIM], in0=a3, in1=s3, op=ALU.mult
            )
            nc.vector.tensor_scalar(
                out=rhs[0:REL_P, :, EMB_DIM : 2 * EMB_DIM],
                in0=xy3, scalar1=0.0, scalar2=None, op0=ALU.is_gt,
            )

            # ---- main loop --------------------------------------------
            scores = outp.tile([128, NT], F32)
            junk = outp.tile([128, EMB_DIM], CDT)

            for t in range(NT):
                ght = ghts[t]
                h1 = ght[:, 0:512]
                h0 = ght[:, 512:1024]
                t1 = ght[:, 1024:1536]
                t0 = ght[:, 1536:2048]

                oh = oht[:, t]
                php = psp.tile([128, EMB_DIM], F32, tag="php")
                mmp = psp.tile([128, EMB_DIM], F32, tag="mmp")
                for c in range(NCHUNK):
                    nc.tensor.matmul(
                        php[:], oh[:, c, :], rhs[:, c, 0:EMB_DIM],
                        start=(c == 0), stop=(c == NCHUNK - 1),
                    )
                for c in range(NCHUNK):
                    nc.tensor.matmul(
                        mmp[:], oh[:, c, :], rhs[:, c, EMB_DIM : 2 * EMB_DIM],
                        start=(c == 0), stop=(c == NCHUNK - 1),
                    )

                mskf = wrk.tile([128, EMB_DIM], CDT, tag="mskf")
                nc.scalar.activation(mskf[:], mmp[:], AF.Copy)
                uv1 = wrk.tile([128, 2, EMB_DIM], CDT, tag="uv1")
                nc.vector.tensor_tensor(out=uv1[:, 0, :], in0=h0, in1=t0,
                                        op=ALU.mult)
                nc.vector.tensor_tensor(out=uv1[:, 1, :], in0=h1, in1=t0,
                                        op=ALU.mult)
                pt1 = wrk.tile([128, EMB_DIM], CDT, tag="pt1")
                nc.vector.tensor_tensor(out=pt1[:], in0=php[:], in1=t1,
                                        op=ALU.mult)
                w2 = wrk.tile([128, 2, EMB_DIM], CDT, tag="w2")
                pt1b = pt1[:].rearrange("p (o d) -> p o d", o=1).broadcast_to(
                    [128, 2, EMB_DIM]
                )
                nc.vector.tensor_tensor(
                    out=w2[:],
                    in0=ght[:, 0:1024].rearrange("p (o d) -> p o d", d=EMB_DIM),
                    in1=pt1b, op=ALU.mult,
                )
                uv = wrk.tile([128, 2, EMB_DIM], CDT, tag="uv")
                nc.vector.tensor_tensor(out=uv[:], in0=uv1[:], in1=w2[:],
                                        op=ALU.add)
                nc.vector.copy_predicated(uv[:, 1, :], mskf[:].bitcast(I16),
                                          uv[:, 0, :])
                nc.scalar.activation(
                    junk[:], uv[:, 1, :], AF.Square,
                    accum_out=scores[:, t : t + 1],
                )

            res = outp.tile([128, NT], F32)
            nc.scalar.activation(res[:], scores[:], AF.Sqrt)
            nc.sync.dma_start(out=out[:], in_=res[:])

    nc.compile()
    return nc


_NC_CACHE = None


def _get_program():
    global _NC_CACHE
    if _NC_CACHE is None:
        _NC_CACHE = build_program()
    return _NC_CACHE


def make_in_maps(head_idx, relation_idx, tail_idx, entity_embedding,
                 relation_embedding, alpha_embedding):
    import ml_dtypes
    NP_F8 = ml_dtypes.float8_e4m3

    head_idx = np.asarray(head_idx).astype(np.int32)
    relation_idx = np.asarray(relation_idx).astype(np.int32)
    tail_idx = np.asarray(tail_idx).astype(np.int32)
    ent = np.asarray(entity_embedding)
    rel = np.asarray(relation_embedding)
    alp = np.asarray(alpha_embedding)

    ea = np.ascontiguousarray(
        ent[:, :, 0, ::-1].transpose(0, 2, 1).reshape(NENTITY, 2 * EMB_DIM)
    ).astype(NP_CDT)

    def chunkmajor(a):  # [500, 512] -> [125, 4*512], relation 125c+p at (p, c)
        return np.ascontiguousarray(
            a.reshape(NCHUNK, REL_P, EMB_DIM).transpose(1, 0, 2)
        ).reshape(REL_P, RX_FREE)

    relx = chunkmajor(rel[:, :, 0] * 128.0).astype(NP_F8)
    rely = chunkmajor(rel[:, :, 1] * 128.0).astype(NP_F8)
    alphaf = chunkmajor(alp * 128.0).astype(NP_F8)


    in_maps = []
    for c in range(NCORES):
        lo = c * B_LOC
        h = head_idx[lo : lo + B_LOC]
        tl = tail_idx[lo : lo + B_LOC]
        r = relation_idx[lo : lo + B_LOC]
        htp = np.empty((128, 2 * NT), np.int32)
        for t in range(NT):
            htp[:, 2 * t] = h[128 * t : 128 * (t + 1)]
            htp[:, 2 * t + 1] = tl[128 * t : 128 * (t + 1)]
        # one-hot: oh[p, t, c, b] = (r[128t+b] == 125c+p)
        rr = r.reshape(NT, 128)
        cc = rr // REL_P          # chunk of each element
        pp = rr % REL_P           # partition within chunk
        oh = np.zeros((128, NT, NCHUNK, 128), NP_F8)
        tt_, bb_ = np.meshgrid(np.arange(NT), np.arange(128), indexing="ij")
        oh[pp, tt_, cc, bb_] = 1.0
        ohv = oh.reshape(128, NT * NCHUNK * 128)
        in_maps.append(
            {
                "ea": ea,
                "relx": relx,
                "rely": rely,
                "alphaf": alphaf,
                "htidx": htp,
                "ohtab": ohv,
            }
        )
    return in_maps


def unshard_out(results):
    full = np.empty(BATCH, np.float32)
    for c in range(NCORES):
        o = np.asarray(results[c]["out"])
        full[c * B_LOC : (c + 1) * B_LOC] = o.T.ravel()
    return full


def kernel(head_idx, relation_idx, tail_idx, entity_embedding,
           relation_embedding, alpha_embedding):
    nc = _get_program()
    in_maps = make_in_maps(head_idx, relation_idx, tail_idx, entity_embedding,
                           relation_embedding, alpha_embedding)
    res = run_bass_kernel_spmd(nc, in_maps, list(range(NCORES)))
    return unshard_out(res.results)


# revision 15
# speedup vs baseline: 2.5301x; 1.0704x over previous
"""PE-one-hot variant: relation planes expanded by TensorE matmuls from an
SBUF-resident table instead of 1024 per-element HBM row gathers.

Same math as kernel.py; the [500 x (phi|m)] table stays in SBUF as a
4-chunk rhs (relation 125c+p at partition p, chunk c).  Per tile the
one-hot matrix oh[p,b] = (relidx[b] == 125c+p) is built with 4 Pool
tensor_scalar compares, and two 4-matmul accumulation groups produce
phi / m planes for the tile's 128 elements in PSUM (exact: one-hot).
"""

import sys

for _p in ("/opt/trn_rl_repo",):
    if _p not in sys.path:
        sys.path.insert(0, _p)

import numpy as np

import concourse.bass as bass
import concourse.bacc as bacc
import concourse.tile as tile
from concourse import mybir
from concourse.bass_utils import run_bass_kernel_spmd

NENTITY, NRELATION, EMB_DIM, K = 200000, 500, 512, 2
BATCH = 8192
NCORES = 8
B_LOC = BATCH // NCORES
NT = B_LOC // 128
CDT = mybir.dt.float16
NP_CDT = np.float16

F32 = mybir.dt.float32
F8 = mybir.dt.float8e4
I16 = mybir.dt.int16
I32 = mybir.dt.int32
AF = mybir.ActivationFunctionType
ALU = mybir.AluOpType

REL_P = 125
NCHUNK = 4
RX_FREE = NCHUNK * EMB_DIM  # 2048, chunk-major relation layout


def build_program():
    nc = bacc.Bacc("TRN2", target_bir_lowering=False, debug=False,
                   num_swdge_queues=4)

    ea = nc.declare_dram_parameter("ea", [NENTITY, 2 * EMB_DIM], CDT, isOutput=False)
    relx = nc.declare_dram_parameter("relx", [REL_P, RX_FREE], F8, isOutput=False)
    rely = nc.declare_dram_parameter("rely", [REL_P, RX_FREE], F8, isOutput=False)
    alphaf = nc.declare_dram_parameter("alphaf", [REL_P, RX_FREE], F8, isOutput=False)
    htidx = nc.declare_dram_parameter("htidx", [128, 2 * NT], I32, isOutput=False)
    ohtab = nc.declare_dram_parameter("ohtab", [128, NT * NCHUNK * 128], F8, isOutput=False)
    out = nc.declare_dram_parameter("out", [128, NT], F32, isOutput=True)

    with tile.TileContext(nc) as tc:
        with (
            tc.tile_pool(name="idx", bufs=1) as idxp,
            tc.tile_pool(name="prep", bufs=1) as prep,
            tc.tile_pool(name="gat", bufs=8) as gat,
            tc.tile_pool(name="oh", bufs=2) as ohp,
            tc.tile_pool(name="psum", bufs=4, space="PSUM") as psp,
            tc.tile_pool(name="wrk", bufs=2) as wrk,
            tc.tile_pool(name="outp", bufs=1) as outp,
        ):
            # ---- uploads (relation tables first; ht index last on the
            # sync ring so gather payload can't starve them)
            ht_t = idxp.tile([128, 2 * NT], I32)
            nc.sync.dma_start(out=ht_t[:], in_=htidx[:])
            alsb = prep.tile([REL_P, RX_FREE], F8)
            nc.scalar.dma_start(out=alsb[:], in_=alphaf[:])
            relxb = prep.tile([REL_P, RX_FREE], F8)
            nc.sync.dma_start(out=relxb[:], in_=relx[:])
            relyb = prep.tile([REL_P, RX_FREE], F8)
            nc.scalar.dma_start(out=relyb[:], in_=rely[:])
            oht = idxp.tile([128, NT, NCHUNK, 128], F8)
            nc.scalar.dma_start(
                out=oht[:].rearrange("p t c b -> p (t c b)"), in_=ohtab[:]
            )

            # preload the sqrt_and_others ACT table set (covers Sign,
            # Square, Copy, Sqrt) so the final Sqrt doesn't swap tables
            # on the critical tail
            sqd = outp.tile([128, 1], F32)
            nc.vector.memset(sqd[:], 1.0)
            nc.scalar.activation(sqd[:], sqd[:], AF.Sqrt)

            # ---- entity gathers ---------------------------------------
            def igather(out_ap, in_ap, off_ap):
                return nc.gpsimd.indirect_dma_start(
                    out=out_ap, out_offset=None, in_=in_ap,
                    in_offset=bass.IndirectOffsetOnAxis(ap=off_ap, axis=0),
                )

            ghts = []
            for t in range(NT):
                ght = gat.tile([128, 2048], CDT, tag="ght")
                igather(ght[:, 0:1024], ea[:], ht_t[:, 2 * t : 2 * t + 1])
                igather(ght[:, 1024:2048], ea[:], ht_t[:, 2 * t + 1 : 2 * t + 2])
                ghts.append(ght)

            # ---- relation-plane table in SBUF -------------------------
            # rhs[p, c, 0:512] = phi for relation 125c+p
            # rhs[p, c, 512:1024] = m   for relation 125c+p
            rhs = prep.tile([128, NCHUNK, 2 * EMB_DIM], F8)
            nc.vector.memset(rhs[:], 0.0)
            asg = prep.tile([REL_P, RX_FREE], CDT)
            nc.scalar.activation(asg[:], alsb[:], AF.Sign)
            xy = prep.tile([REL_P, RX_FREE], CDT)
            nc.vector.tensor_tensor(out=xy[:], in0=relxb[:], in1=relyb[:],
                                    op=ALU.mult)
            sxy = prep.tile([REL_P, RX_FREE], CDT)
            nc.scalar.activation(sxy[:], xy[:], AF.Sign)
            a3 = asg[:].rearrange("p (c d) -> p c d", d=EMB_DIM)
            s3 = sxy[:].rearrange("p (c d) -> p c d", d=EMB_DIM)
            xy3 = xy[:].rearrange("p (c d) -> p c d", d=EMB_DIM)
            nc.vector.tensor_tensor(
                out=rhs[0:REL_P, :, 0:EMB_DIM], in0=a3, in1=s3, op=ALU.mult
            )
            nc.vector.tensor_scalar(
                out=rhs[0:REL_P, :, EMB_DIM : 2 * EMB_DIM],
                in0=xy3, scalar1=0.0, scalar2=None, op0=ALU.is_gt,
            )

            # ---- main loop --------------------------------------------
            scores = outp.tile([128, NT], F32)
            junk = outp.tile([128, EMB_DIM], CDT)

            for t in range(NT):
                ght = ghts[t]
                h1 = ght[:, 0:512]
                h0 = ght[:, 512:1024]
                t1 = ght[:, 1024:1536]
                t0 = ght[:, 1536:2048]

                oh = oht[:, t]
                php = psp.tile([128, EMB_DIM], F32, tag="php")
                mmp = psp.tile([128, EMB_DIM], F32, tag="mmp")
                for c in range(NCHUNK):
                    nc.tensor.matmul(
                        php[:], oh[:, c, :], rhs[:, c, 0:EMB_DIM],
                        start=(c == 0), stop=(c == NCHUNK - 1),
                    )
                for c in range(NCHUNK):
                    nc.tensor.matmul(
                        mmp[:], oh[:, c, :], rhs[:, c, EMB_DIM : 2 * EMB_DIM],
                        start=(c == 0), stop=(c == NCHUNK - 1),
                    )

                mskf = wrk.tile([128, EMB_DIM], CDT, tag="mskf")
                nc.scalar.activation(mskf[:], mmp[:], AF.Copy)
                phs = wrk.tile([128, EMB_DIM], CDT, tag="phs")
                nc.scalar.activation(phs[:], php[:], AF.Copy)
                uv1 = wrk.tile([128, 2, EMB_DIM], CDT, tag="uv1")
                nc.vector.tensor_tensor(out=uv1[:, 0, :], in0=h0, in1=t0,
                                        op=ALU.mult)
                nc.vector.tensor_tensor(out=uv1[:, 1, :], in0=h1, in1=t0,
                                        op=ALU.mult)
                pt1 = wrk.tile([128, EMB_DIM], CDT, tag="pt1")
                nc.vector.tensor_tensor(out=pt1[:], in0=phs[:], in1=t1,
                                        op=ALU.mult)
                w2 = wrk.tile([128, 2, EMB_DIM], CDT, tag="w2")
                pt1b = pt1[:].rearrange("p (o d) -> p o d", o=1).broadcast_to(
                    [128, 2, EMB_DIM]
                )
                nc.vector.tensor_tensor(
                    out=w2[:],
                    in0=ght[:, 0:1024].rearrange("p (o d) -> p o d", d=EMB_DIM),
                    in1=pt1b, op=ALU.mult,
                )
                uv = wrk.tile([128, 2, EMB_DIM], CDT, tag="uv")
                nc.vector.tensor_tensor(out=uv[:], in0=uv1[:], in1=w2[:],
                                        op=ALU.add)
                nc.vector.copy_predicated(uv[:, 1, :], mskf[:].bitcast(I16),
                                          uv[:, 0, :])
                nc.scalar.activation(
                    junk[:], uv[:, 1, :], AF.Square,
                    accum_out=scores[:, t : t + 1],
                )

            res = outp.tile([128, NT], F32)
            nc.scalar.activation(res[:], scores[:], AF.Sqrt)
            nc.sync.dma_start(out=out[:], in_=res[:])

    nc.compile()
    return nc


_NC_CACHE = None


def _get_program():
    global _NC_CACHE
    if _NC_CACHE is None:
        _NC_CACHE = build_program()
    return _NC_CACHE


def make_in_maps(head_idx, relation_idx, tail_idx, entity_embedding,
                 relation_embedding, alpha_embedding):
    import ml_dtypes
    NP_F8 = ml_dtypes.float8_e4m3

    head_idx = np.asarray(head_idx).astype(np.int32)
    relation_idx = np.asarray(relation_idx).astype(np.int32)
    tail_idx = np.asarray(tail_idx).astype(np.int32)
    ent = np.asarray(entity_embedding)
    rel = np.asarray(relation_embedding)
    alp = np.asarray(alpha_embedding)

    ea = np.ascontiguousarray(
        ent[:, :, 0, ::-1].transpose(0, 2, 1).reshape(NENTITY, 2 * EMB_DIM)
    ).astype(NP_CDT)

    def chunkmajor(a):  # [500, 512] -> [125, 4*512], relation 125c+p at (p, c)
        return np.ascontiguousarray(
            a.reshape(NCHUNK, REL_P, EMB_DIM).transpose(1, 0, 2)
        ).reshape(REL_P, RX_FREE)

    relx = chunkmajor(rel[:, :, 0] * 128.0).astype(NP_F8)
    rely = chunkmajor(rel[:, :, 1] * 128.0).astype(NP_F8)
    alphaf = chunkmajor(alp * 128.0).astype(NP_F8)


    in_maps = []
    for c in range(NCORES):
        lo = c * B_LOC
        h = head_idx[lo : lo + B_LOC]
        tl = tail_idx[lo : lo + B_LOC]
        r = relation_idx[lo : lo + B_LOC]
        htp = np.empty((128, 2 * NT), np.int32)
        for t in range(NT):
            htp[:, 2 * t] = h[128 * t : 128 * (t + 1)]
            htp[:, 2 * t + 1] = tl[128 * t : 128 * (t + 1)]
        # one-hot: oh[p, t, c, b] = (r[128t+b] == 125c+p)
        rr = r.reshape(NT, 128)
        cc = rr // REL_P          # chunk of each element
        pp = rr % REL_P           # partition within chunk
        oh = np.zeros((128, NT, NCHUNK, 128), NP_F8)
        tt_, bb_ = np.meshgrid(np.arange(NT), np.arange(128), indexing="ij")
        oh[pp, tt_, cc, bb_] = 1.0
        ohv = oh.reshape(128, NT * NCHUNK * 128)
        in_maps.append(
            {
                "ea": ea,
                "relx": relx,
                "rely": rely,
                "alphaf": alphaf,
                "htidx": htp,
                "ohtab": ohv,
            }
        )
    return in_maps


def unshard_out(results):
    full = np.empty(BATCH, np.float32)
    for c in range(NCORES):
        o = np.asarray(results[c]["out"])
        full[c * B_LOC : (c + 1) * B_LOC] = o.T.ravel()
    return full


def kernel(head_idx, relation_idx, tail_idx, entity_embedding,
           relation_embedding, alpha_embedding):
    nc = _get_program()
    in_maps = make_in_maps(head_idx, relation_idx, tail_idx, entity_embedding,
                           relation_embedding, alpha_embedding)
    res = run_bass_kernel_spmd(nc, in_maps, list(range(NCORES)))
    return unshard_out(res.results)
